# revision 20
# baseline (speedup 1.0000x reference)
"""Trainium2 Bass kernel for a 4-layer dense transformer encoder.

Problem: nn_Encoder (LAYERS=4, D_MODEL=1024, HEADS=16, HIDDEN=4096, B=2, L=2048).

Sharding: 8 cores x 512 tokens (cores 0-3 batch 0, 4-7 batch 1). Weights
replicated. Per layer each core projects K/V for its own tokens; K/V shards are
all-gathered in fp8 within each 4-core group in two token halves so gathers
overlap compute. The layer boundary is software-pipelined: FFN2 runs in token
halves and each half's K/V projection + gather for the NEXT layer issues
immediately, so the gather latency hides behind the rest of FFN2, the Q
projection and the first attention phase.

Precision: residual stream fp32; scores path bf16 (Q, K dequantized);
V / exp(scores) / ctx / g and the Wout/W2 weights are fp8e4m3 with DoubleRow
matmuls (2x PE throughput, contract=2x128). fp8 weights are pre-scaled x32 on
the host (values ~N(0,0.02^2) would land in the subnormal range) and the 1/32
is folded into the bias-add / gelu scale. exp uses bias -2 so e^max stays
within fp8 range; the softmax denominator (ones column appended to V) is
unaffected by the shift. Softmax division is batched: one DVE reciprocal over
all 16 head denominators per layer instead of 64 slow per-head reciprocals.
"""

import numpy as np
import ml_dtypes

import concourse.bass as bass
import concourse.mybir as mybir
import concourse.tile as tile
from concourse import bacc
from concourse.bass_utils import run_bass_kernel_spmd

F32 = mybir.dt.float32
BF16 = mybir.dt.bfloat16
F8 = mybir.dt.float8e4
AF = mybir.ActivationFunctionType
DR = mybir.MatmulPerfMode.DoubleRow

LAYERS, D, HEADS, DK, HID = 4, 1024, 16, 64, 4096
B, L = 2, 2048
P = 128
TOK = 512          # tokens per core
HTOK = 256         # tokens per gather half
FT = D // P        # 8 feature tiles
HT = HID // P      # 32 hidden tiles
RANKS = 4          # cores per gather group
VE = DK + 1        # 65: per-head V columns + ones column
NP = HEADS // 2    # 8 head pairs
KSZ = FT * P * HTOK            # K staging elems per half [8,128,256]
VSZ = P * 2 * HEADS * VE       # V staging elems per half [128,2,16,65]
CH = KSZ + VSZ
N_CORES = 8
WSC = 32.0                     # fp8 weight pre-scale
EC = 2.0                       # exp bias: e' = exp(s/8 - EC)

_CACHE = {}


def build_nc(layers=LAYERS):
    nc = bacc.Bacc("TRN2", target_bir_lowering=False, debug=False,
                   num_devices=N_CORES)
    LY = layers
    x_fm = nc.dram_tensor("x_fm", [FT, P, TOK], F32, kind="ExternalInput").ap()
    # Q/K weights bf16: [LY, 16, P, FT, P]; nt 0-7 = Q tiles, 8-15 = K tiles
    wqk = nc.dram_tensor("wqk", [LY, 16, P, FT, P], BF16, kind="ExternalInput").ap()
    wv = nc.dram_tensor("wv", [LY, P, FT, D], BF16, kind="ExternalInput").ap()
    # fp8 DoubleRow weights (pre-scaled x32)
    wout = nc.dram_tensor("wout", [LY, FT, DK, NP, 2, P], F8, kind="ExternalInput").ap()
    w1 = nc.dram_tensor("w1", [LY, HT, P, FT, P], BF16, kind="ExternalInput").ap()
    w2 = nc.dram_tensor("w2", [LY, FT, P, HT, P], BF16, kind="ExternalInput").ap()
    bqk = nc.dram_tensor("bqk", [LY, P, 16], F32, kind="ExternalInput").ap()
    bv = nc.dram_tensor("bv", [LY, 1, D], BF16, kind="ExternalInput").ap()
    bout = nc.dram_tensor("bout", [LY, P, FT], F32, kind="ExternalInput").ap()
    b1 = nc.dram_tensor("b1", [LY, P, HT], F32, kind="ExternalInput").ap()
    b2 = nc.dram_tensor("b2", [LY, P, FT], F32, kind="ExternalInput").ap()
    mskd = nc.dram_tensor("mskd", [HEADS, NP, 2, DK], BF16, kind="ExternalInput").ap()
    out_fm = nc.dram_tensor("out_fm", [FT, P, TOK], F32, kind="ExternalOutput").ap()

    groups = [[0, 1, 2, 3], [4, 5, 6, 7]]

    from contextlib import ExitStack

    with tile.TileContext(nc) as tc:
        with ExitStack() as stack:
            pool = lambda name, bufs, **kw: stack.enter_context(
                tc.tile_pool(name=name, bufs=bufs, **kw))
            hp = pool("hp", 1)          # residual stream fp32 (in-place)
            hbfp = pool("hbfp", 1)      # bf16 matmul input (h, then h_mid)
            qp = pool("qp", 1)
            gp = pool("gp", 1)          # gelu output fp8
            cap = pool("cap", 1)        # parked ctx partials bf16 (per half)
            ctfp = pool("ctfp", 2)      # ctx A+B sum fp32
            cxp = pool("cxp", 1)        # normalized ctx fp8 [64,NP,2,TOK]
            denp = pool("denp", 1)
            vallp = pool("vallp", 2)
            khp = pool("khp", 2)
            khbp = pool("khbp", 2)
            ep = pool("ep", 2)
            wqp = pool("wqp", 2)
            wkp = pool("wkp", 1)        # K proj weights, full-layer tile
            wvp = pool("wvp", 1)
            wop = pool("wop", 2)
            w1p = pool("w1p", 2)
            w2p = pool("w2p", 3)
            biasp = pool("biasp", 2)
            kvsp = pool("kvsp", 2)
            t1p = pool("t1p", 2)
            constp = pool("constp", 1)
            psp0 = pool("psp0", 1, space="PSUM")
            psp1 = pool("psp1", 1, space="PSUM")
            pcsp = pool("pcsp", 1, space="PSUM")
            up = pool("up", 2, space="PSUM")
            dramp = pool("dramp", 2, space="DRAM")

            ones_bf = constp.tile([1, P], BF16)
            nc.vector.memset(ones_bf[:], 1.0)
            negc = constp.tile([P, 1], F32)
            nc.vector.memset(negc[:], -EC)
            # Msk2[c, p, s, j] = 1 iff c == 2p+s : selects a head's 1/denom
            # row and broadcasts it to 64 partitions via one small matmul.
            msk = constp.tile([HEADS, NP, 2, DK], BF16)
            nc.sync.dma_start(msk[:], mskd)

            h = hp.tile([P, FT, TOK], F32, tag="h", name="h0")
            nc.sync.dma_start(h[:], x_fm.rearrange("a p t -> p a t"))
            h_bf = hbfp.tile([P, FT, TOK], BF16, tag="hbf", name="hbf0")
            nc.vector.tensor_copy(out=h_bf[:], in_=h[:])

            def load_kv_weights(ly):
                wk_sb = wkp.tile([P, FT, FT, P], BF16, tag="wk", name=f"wk_{ly}")
                nc.sync.dma_start(wk_sb[:], wqk[ly, FT:16].rearrange("n p a b -> p n a b"))
                wv_sb = wvp.tile([P, FT, D], BF16, tag="wv", name=f"wv_{ly}")
                nc.sync.dma_start(wv_sb[:], wv[ly])
                bqk_sb = biasp.tile([P, 16], F32, tag="bqk", name=f"bqk_{ly}")
                nc.sync.dma_start(bqk_sb[:], bqk[ly])
                bv_sb = biasp.tile([1, D], BF16, tag="bv", name=f"bv_{ly}")
                nc.sync.dma_start(bv_sb[:], bv[ly])
                return wk_sb, wv_sb, bqk_sb, bv_sb

            def kv_half(ly, half, h_bf_c, wk_sb, wv_sb, bqk_sb, bv_sb):
                """K (feature-major) + V (token-major +ones) fp8 -> staging."""
                t0 = half * HTOK
                kv_stage = dramp.tile([CH], F8, tag=f"stage{half}",
                                      name=f"stage_{ly}_{half}")
                k_view = kv_stage[0:KSZ].rearrange("(a p t) -> a p t", p=P, t=HTOK)
                v_view = kv_stage[KSZ:CH].rearrange("(p a h e) -> p a h e",
                                                    a=2, h=HEADS, e=VE)
                for nt in range(FT):
                    pp = up.tile([P, TOK], F32, tag="u", name=f"ppk_{ly}_{half}_{nt}")
                    for kt in range(FT):
                        nc.tensor.matmul(pp[:, 0:HTOK], lhsT=wk_sb[:, nt, kt, :],
                                         rhs=h_bf_c[:, kt, t0:t0 + HTOK],
                                         start=(kt == 0), stop=(kt == FT - 1))
                    k_sb = kvsp.tile([P, HTOK], F8, tag="ksb",
                                     name=f"ksb_{ly}_{half}_{nt}")
                    nc.vector.tensor_scalar_add(k_sb[:], pp[:, 0:HTOK],
                                                bqk_sb[:, FT + nt:FT + nt + 1])
                    nc.sync.dma_start(k_view[nt], k_sb[:])
                for tt in range(2):
                    for ch in range(2):
                        pp = up.tile([P, TOK], F32, tag="u",
                                     name=f"ppv_{ly}_{half}_{tt}_{ch}")
                        for kt in range(FT):
                            nc.tensor.matmul(
                                pp[:], lhsT=h_bf_c[:, kt, t0 + tt * P:t0 + tt * P + P],
                                rhs=wv_sb[:, kt, ch * 512:(ch + 1) * 512],
                                start=(kt == 0), stop=False)
                        nc.tensor.matmul(pp[:], lhsT=ones_bf[:],
                                         rhs=bv_sb[:, ch * 512:(ch + 1) * 512],
                                         start=False, stop=True)
                        v_sb = kvsp.tile([P, FT, VE], F8, tag="vsb",
                                         name=f"vsb_{ly}_{half}_{tt}_{ch}")
                        nc.vector.tensor_copy(
                            out=v_sb[:, :, 0:DK],
                            in_=pp[:].rearrange("p (a b) -> p a b", b=DK))
                        nc.vector.memset(v_sb[:, :, DK:VE], 1.0)
                        nc.sync.dma_start(
                            v_view[:, tt, ch * FT:(ch + 1) * FT, :], v_sb[:])
                return kv_stage

            def all_gather(ly, half, kv_stage):
                kv_gath = dramp.tile([RANKS * CH], F8, tag=f"gath{half}",
                                     name=f"gath_{ly}_{half}")
                nc.gpsimd.collective_compute(
                    "AllGather", mybir.AluOpType.bypass, replica_groups=groups,
                    ins=[kv_stage.opt()], outs=[kv_gath.opt()])
                return kv_gath

            def q_proj(ly, h_bf_c, bqk_sb):
                q_sb = qp.tile([P, FT, TOK], BF16, tag="q", name=f"q_{ly}")
                for nt in range(FT):
                    wt = wqp.tile([P, FT, P], BF16, tag="wq", name=f"wq_{ly}_{nt}")
                    nc.sync.dma_start(wt[:], wqk[ly, nt])
                    pp = up.tile([P, TOK], F32, tag="u", name=f"ppq_{ly}_{nt}")
                    for kt in range(FT):
                        nc.tensor.matmul(pp[:], lhsT=wt[:, kt, :],
                                         rhs=h_bf_c[:, kt, :],
                                         start=(kt == 0), stop=(kt == FT - 1))
                    nc.vector.tensor_scalar_add(q_sb[:, nt, :], pp[:],
                                                bqk_sb[:, nt:nt + 1])
                return q_sb

            # ---------- prologue: layer 0 K/V + gathers + Q ----------
            wk_sb, wv_sb, bqk_sb, bv_sb = load_kv_weights(0)
            stage_a = kv_half(0, 0, h_bf, wk_sb, wv_sb, bqk_sb, bv_sb)
            gath = [all_gather(0, 0, stage_a), None]
            stage_b = kv_half(0, 1, h_bf, wk_sb, wv_sb, bqk_sb, bv_sb)
            gath[1] = all_gather(0, 1, stage_b)
            q_sb = q_proj(0, h_bf, bqk_sb)

            for ly in range(layers):
                # per-layer biases
                bout_sb = biasp.tile([P, FT], F32, tag="bout", name=f"bout_{ly}")
                nc.sync.dma_start(bout_sb[:], bout[ly])
                b1_sb = biasp.tile([P, HT], F32, tag="b1", name=f"b1_{ly}")
                nc.sync.dma_start(b1_sb[:], b1[ly])
                b2_sb = biasp.tile([P, FT], F32, tag="b2", name=f"b2_{ly}")
                nc.sync.dma_start(b2_sb[:], b2[ly])

                # ---------- attention ----------
                ca = [None, None]   # parked ctx+denom per half, bf16
                dn = [None, None]
                for half in range(2):
                    if half == 1 and ly + 1 < layers:
                        # prefetch next layer's K/V weights under half-0 cover
                        wk_n, wv_n, bqk_n, bv_n = load_kv_weights(ly + 1)
                    rk = gath[half].rearrange("(r c) -> r c", c=CH)
                    v_all = vallp.tile([P, RANKS, 2, HEADS, VE], F8, tag="vall",
                                       name=f"vall_{ly}_{half}")
                    for r in range(RANKS):
                        nc.sync.dma_start(
                            v_all[:, r, :, :, :],
                            rk[r, KSZ:CH].rearrange("(p a h e) -> p a h e",
                                                    a=2, h=HEADS, e=VE))
                    cah = cap.tile([VE, NP, 2, TOK], BF16, tag=f"ca{half}",
                                   name=f"ca_{ly}_{half}")
                    ca[half] = cah

                    def emit_scores(kh_s, hpair_s, r):
                        """Scores for rank-chunk r; the two subs' matmuls are
                        adjacent in the PE queue so the 64-contract row tiles
                        (bases 0/64) execute concurrently."""
                        pS = []
                        for sub in range(2):
                            ps_pool = psp0 if sub == 0 else psp1
                            pS.append(ps_pool.tile(
                                [P, 2, TOK], F32, tag=f"ps{sub}",
                                name=f"ps_{ly}_{half}_{hpair_s}_{r}_{sub}"))
                        for tt in range(2):
                            for sub in range(2):
                                base = sub * DK
                                nc.tensor.matmul(
                                    pS[sub][:, tt, :],
                                    lhsT=kh_s[base:base + DK, r, tt * P:(tt + 1) * P],
                                    rhs=q_sb[base:base + DK, hpair_s, :],
                                    start=True, stop=True)
                        return pS

                    for hpair in range(NP):
                        kh8 = khp.tile([P, RANKS, HTOK], F8, tag="kh",
                                       name=f"kh_{ly}_{half}_{hpair}")
                        ksrc = rk[:, hpair * (P * HTOK):(hpair + 1) * (P * HTOK)]
                        nc.sync.dma_start(
                            kh8[:], ksrc.rearrange("r (p t) -> p r t", t=HTOK))
                        kh = khbp.tile([P, RANKS, HTOK], BF16, tag="khb",
                                       name=f"khb_{ly}_{half}_{hpair}")
                        nc.vector.tensor_copy(out=kh[:], in_=kh8[:])

                        pcs = pcsp.tile([P, 2, TOK], F32, tag="pcs",
                                        name=f"pcs_{ly}_{half}_{hpair}")
                        pS_cur = emit_scores(kh, hpair, 0)
                        pS_nxt = emit_scores(kh, hpair, 1)
                        for r in range(RANKS):
                            e_sb = []
                            for sub in range(2):
                                e_s = ep.tile([P, 2, TOK], F8, tag="e",
                                              name=f"e_{ly}_{half}_{hpair}_{r}_{sub}")
                                nc.scalar.activation(e_s[:], pS_cur[sub][:],
                                                     AF.Exp, scale=0.125,
                                                     bias=negc[:])
                                e_sb.append(e_s)
                            for sub in range(2):
                                hd = 2 * hpair + sub
                                nc.tensor.matmul(
                                    pcs[0:VE, sub, :],
                                    lhsT=v_all[:, r, :, hd, :],
                                    rhs=e_sb[sub][:],
                                    start=(r == 0), stop=(r == RANKS - 1),
                                    perf_mode=DR)
                            if r + 2 < RANKS:
                                pS_cur, pS_nxt = pS_nxt, emit_scores(kh, hpair, r + 2)
                            else:
                                pS_cur, pS_nxt = pS_nxt, None
                        # park ctx partial + denominator row (bf16, one copy)
                        nc.vector.tensor_copy(out=cah[0:VE, hpair, :, :],
                                              in_=pcs[0:VE, :, :])
                    # gather this half's 16 denominator rows partition-major
                    dnh = denp.tile([HEADS, TOK], BF16, tag=f"dn{half}",
                                    name=f"dn_{ly}_{half}")
                    nc.sync.dma_start(dnh[:], cah[DK:DK + 1, :, :, :])
                    dn[half] = dnh

                # ---------- batched softmax finalize ----------
                den = denp.tile([HEADS, TOK], F32, tag="den", name=f"den_{ly}")
                nc.vector.tensor_add(out=den[:], in0=dn[0][:], in1=dn[1][:])
                rec = denp.tile([HEADS, TOK], F32, tag="rec", name=f"rec_{ly}")
                nc.vector.reciprocal(rec[:], den[:])
                drb = denp.tile([HEADS, TOK], BF16, tag="drb", name=f"drb_{ly}")
                nc.vector.tensor_copy(out=drb[:], in_=rec[:])
                ctx8 = cxp.tile([DK, NP, 2, TOK], F8, tag="cx", name=f"cx_{ly}")
                for p_ in range(NP):
                    pbt = psp0.tile([P, 2, TOK], F32, tag="ps0", name=f"pb_{ly}_{p_}")
                    for s_ in range(2):
                        nc.tensor.matmul(pbt[0:DK, s_, :], lhsT=msk[:, p_, s_, :],
                                         rhs=drb[:], start=True, stop=True)
                    csum = ctfp.tile([DK, 2, TOK], BF16, tag="ctf",
                                     name=f"ctf_{ly}_{p_}")
                    nc.vector.tensor_add(out=csum[:], in0=ca[0][0:DK, p_, :, :],
                                         in1=ca[1][0:DK, p_, :, :])
                    nc.vector.tensor_tensor(out=ctx8[:, p_, :, :], in0=csum[:],
                                            in1=pbt[0:DK, :, :],
                                            op=mybir.AluOpType.mult)

                # ---------- output projection (fp8 DR) + residual ----------
                hm_bf = hbfp.tile([P, FT, TOK], BF16, tag="hbf", name=f"hmbf_{ly}")
                for nt in range(FT):
                    wo = wop.tile([DK, NP, 2, P], F8, tag="wo", name=f"wo_{ly}_{nt}")
                    nc.sync.dma_start(wo[:], wout[ly, nt])
                    pp = up.tile([P, TOK], F32, tag="u", name=f"ppo_{ly}_{nt}")
                    for p_ in range(NP):
                        nc.tensor.matmul(pp[:], lhsT=wo[:, p_, :, :],
                                         rhs=ctx8[:, p_, :, :],
                                         start=(p_ == 0), stop=(p_ == NP - 1),
                                         perf_mode=DR)
                    t1 = t1p.tile([P, TOK], F32, tag="t1", name=f"t1o_{ly}_{nt}")
                    nc.vector.tensor_scalar(out=t1[:], in0=pp[:],
                                            scalar1=1.0 / WSC,
                                            scalar2=bout_sb[:, nt:nt + 1],
                                            op0=mybir.AluOpType.mult,
                                            op1=mybir.AluOpType.add)
                    nc.vector.tensor_add(out=h[:, nt, :], in0=t1[:],
                                         in1=h[:, nt, :])
                    nc.vector.tensor_copy(out=hm_bf[:, nt, :], in_=h[:, nt, :])

                # ---------- FFN1 (bf16) + gelu -> g bf16 ----------
                g = gp.tile([P, HT, TOK], BF16, tag="g", name=f"g_{ly}")
                for nt in range(HT):
                    wt = w1p.tile([P, FT, P], BF16, tag="w1", name=f"w1_{ly}_{nt}")
                    nc.sync.dma_start(wt[:], w1[ly, nt])
                    pp = up.tile([P, TOK], F32, tag="u", name=f"ppf_{ly}_{nt}")
                    for kt in range(FT):
                        nc.tensor.matmul(pp[:], lhsT=wt[:, kt, :],
                                         rhs=hm_bf[:, kt, :],
                                         start=(kt == 0), stop=(kt == FT - 1))
                    nc.scalar.activation(g[:, nt, :], pp[:], AF.Gelu,
                                         bias=b1_sb[:, nt:nt + 1])

                # ---------- FFN2 (fp8 DR) by token halves + next-layer KV ----
                last = ly + 1 >= layers
                if not last:
                    hn_bf = hbfp.tile([P, FT, TOK], BF16, tag="hbf",
                                      name=f"hnbf_{ly}")
                for half in range(2):
                    t0 = half * HTOK
                    for nt in range(FT):
                        pp = up.tile([P, TOK], F32, tag="u",
                                     name=f"pf2_{ly}_{half}_{nt}")
                        for hh in range(2):
                            w2t = w2p.tile([P, HT // 2, P], BF16, tag="w2",
                                           name=f"w2_{ly}_{half}_{nt}_{hh}")
                            nc.sync.dma_start(
                                w2t[:], w2[ly, nt, :, hh * 16:(hh + 1) * 16, :])
                            for k2 in range(HT // 2):
                                kt = hh * 16 + k2
                                nc.tensor.matmul(
                                    pp[:, 0:HTOK], lhsT=w2t[:, k2, :],
                                    rhs=g[:, kt, t0:t0 + HTOK],
                                    start=(kt == 0), stop=(kt == HT - 1))
                        t1 = t1p.tile([P, TOK], F32, tag="t1",
                                      name=f"t1f_{ly}_{half}_{nt}")
                        nc.vector.tensor_scalar_add(t1[:, 0:HTOK], pp[:, 0:HTOK],
                                                    b2_sb[:, nt:nt + 1])
                        nc.vector.tensor_add(out=h[:, nt, t0:t0 + HTOK],
                                             in0=t1[:, 0:HTOK],
                                             in1=h[:, nt, t0:t0 + HTOK])
                        if not last:
                            nc.vector.tensor_copy(out=hn_bf[:, nt, t0:t0 + HTOK],
                                                  in_=h[:, nt, t0:t0 + HTOK])
                    if not last:
                        stg = kv_half(ly + 1, half, hn_bf, wk_n, wv_n,
                                      bqk_n, bv_n)
                        gath[half] = all_gather(ly + 1, half, stg)
                if not last:
                    q_sb = q_proj(ly + 1, hn_bf, bqk_n)

            nc.sync.dma_start(out_fm.rearrange("a p t -> p a t"), h[:])
    nc.compile()
    return nc


def _prep_inputs(x, Wqkv, bqkv, Wout, bout, W1, b1, W2, b2, layers=LAYERS):
    """Host-side re-tiling of the full inputs into per-core in_maps."""
    bf = ml_dtypes.bfloat16
    f8 = ml_dtypes.float8_e4m3fn
    x = np.asarray(x, dtype=np.float32)
    Wqkv = np.asarray(Wqkv, dtype=np.float32)
    bqkv = np.asarray(bqkv, dtype=np.float32)
    Wout_ = np.asarray(Wout, dtype=np.float32)
    bout_ = np.asarray(bout, dtype=np.float32)
    W1_ = np.asarray(W1, dtype=np.float32)
    b1_ = np.asarray(b1, dtype=np.float32)
    W2_ = np.asarray(W2, dtype=np.float32)
    b2_ = np.asarray(b2, dtype=np.float32)
    LY = layers

    wqk = np.ascontiguousarray(
        Wqkv[:LY, :, :2 * D].reshape(LY, FT, P, 16, P).transpose(0, 3, 2, 1, 4)
    ).astype(bf)
    wv = np.ascontiguousarray(
        Wqkv[:LY, :, 2 * D:].reshape(LY, FT, P, D).transpose(0, 2, 1, 3)
    ).astype(bf)
    # fp8 DR layouts (pre-scaled x32)
    wout = np.ascontiguousarray(
        (Wout_[:LY] * WSC).reshape(LY, NP, 2, DK, FT, P).transpose(0, 4, 3, 1, 2, 5)
    ).astype(f8)
    w1 = np.ascontiguousarray(
        W1_[:LY].reshape(LY, FT, P, HT, P).transpose(0, 3, 2, 1, 4)
    ).astype(bf)
    w2 = np.ascontiguousarray(
        W2_[:LY].reshape(LY, HT, P, FT, P).transpose(0, 3, 2, 1, 4)
    ).astype(bf)
    bqkt = np.ascontiguousarray(
        bqkv[:LY, :2 * D].reshape(LY, 16, P).transpose(0, 2, 1))
    bvv = bqkv[:LY, 2 * D:].reshape(LY, 1, D).astype(bf)
    boutt = np.ascontiguousarray(bout_[:LY].reshape(LY, FT, P).transpose(0, 2, 1))
    b1t = np.ascontiguousarray(b1_[:LY].reshape(LY, HT, P).transpose(0, 2, 1))
    b2t = np.ascontiguousarray(b2_[:LY].reshape(LY, FT, P).transpose(0, 2, 1))

    mskh = np.zeros((HEADS, NP, 2, DK), dtype=bf)
    for p_ in range(NP):
        for s_ in range(2):
            mskh[2 * p_ + s_, p_, s_, :] = 1.0

    shared = dict(wqk=wqk, wv=wv, wout=wout, w1=w1, w2=w2, bqk=bqkt, bv=bvv,
                  bout=boutt, b1=b1t, b2=b2t, mskd=mskh)
    in_maps = []
    for c in range(N_CORES):
        b, r = divmod(c, RANKS)
        xc = x[b, r * TOK:(r + 1) * TOK, :]          # [512, 1024]
        x_fm = np.ascontiguousarray(xc.T).reshape(FT, P, TOK)
        in_maps.append({"x_fm": x_fm, **shared})
    return in_maps


def kernel(x, Wqkv, bqkv, Wout, bout, W1, b1, W2, b2):
    if "nc" not in _CACHE:
        _CACHE["nc"] = build_nc()
    nc = _CACHE["nc"]
    in_maps = _prep_inputs(x, Wqkv, bqkv, Wout, bout, W1, b1, W2, b2)
    res = run_bass_kernel_spmd(nc, in_maps, core_ids=list(range(N_CORES)))
    out = np.empty((B, L, D), dtype=np.float32)
    for c in range(N_CORES):
        b, r = divmod(c, RANKS)
        o = res.results[c]["out_fm"].reshape(D, TOK)      # [1024, 512]
        out[b, r * TOK:(r + 1) * TOK, :] = o.T
    return out
